# revision 36
# baseline (speedup 1.0000x reference)
"""Trainium2 Bass kernel for attention-pooling (AttLayer).

Computes, per batch row b:
    z   = x[b] @ W + bias            # [S, A]
    t   = tanh(z)
    sc  = t @ u                      # [S]
    e   = exp(sc) * mask[b]
    out = (x[b]^T @ e) / (sum(e) + 1e-7)   # [D]

Sharding: data-parallel over batch across 8 NeuronCores (8 rows each).

Design (v5):
- Host compacts unmasked positions per row (~50% dense mask) and zero-pads.
  Padding rows of x are zero, so they contribute nothing to the numerator;
  the denominator is computed on host from the returned e row with the
  compacted mask. No mask work on device at all.
- Jagged slots: batches are sorted by unmasked count and distributed so
  slot j holds similar-count batches on every core. Each slot gets its own
  compacted length S_c[j] (64-multiple), so most slots run with 1024
  columns (two clean 512-wide matmul blocks, no tail) instead of the
  global max. One compile per slot-length tuple.
- x is streamed ONCE per batch in a transposed layout xt[p, dc*S_c+s] =
  x[s, dc*128+p], host-packed so each SBUF partition line is one
  contiguous DMA descriptor.
- Stage 1 (weight-major): z^T accumulated in a merged [128, <=1024] psum
  tile plus optional tail bank; tanh with per-partition bias runs as two
  big ScalarE instructions per a-chunk.
- Stage 2 uses a column-replicated u as lhsT (m=128, same cycle count as
  m=1), so exp(score) lands broadcast across all 128 partitions in one
  merged psum tile; a single Exp per batch.
- Weighted sum: tensor_mul on VectorE (bf16 2x) per d-chunk; reduction via
  a short fold tree on VectorE for 3 chunks and a Copy+accum_out on
  ScalarE for the 4th, balancing engine load.
- Host: out = num / (sum(e * maskc) + EPS), un-permuted.
"""

import os
import numpy as np
import ml_dtypes

B, S, D, A = 64, 2048, 512, 256
NCORES = 8
BL = B // NCORES          # batches per core
NDC = D // 128            # 4 d-chunks
NAC = A // 128            # 2 a-chunks
EPS = 1e-7

_cache = {}
last_results = None       # BassKernelResults of the most recent run


def _blocks_of(S_c):
    """Split S_c into seq blocks of at most 512."""
    out = []
    rem = S_c
    while rem > 0:
        blk = min(512, rem)
        out.append(blk)
        rem -= blk
    return out


def _build_bass(sc_list):
    import concourse.mybir as mybir
    import concourse.tile as tile
    from concourse import bacc

    f32 = mybir.dt.float32
    bf16 = mybir.dt.bfloat16
    AF = mybir.ActivationFunctionType

    assert len(sc_list) == BL
    S_cmax = max(sc_list)
    assert all(sc % 64 == 0 for sc in sc_list)

    nc = bacc.Bacc()

    xt = nc.declare_dram_parameter("xt", [BL, 128, NDC * S_cmax], bf16, isOutput=False)
    w2 = nc.declare_dram_parameter("w2", [128, NDC * A], bf16, isOutput=False)
    u2 = nc.declare_dram_parameter("u2", [128, NAC * 128], bf16, isOutput=False)
    b2 = nc.declare_dram_parameter("b2", [128, NAC], f32, isOutput=False)
    num = nc.declare_dram_parameter("num", [128, BL * NDC], f32, isOutput=True)
    eo = nc.declare_dram_parameter("eo", [BL, 1, S_cmax], bf16, isOutput=True)

    with tile.TileContext(nc) as tc:
        with (
            tc.tile_pool(name="consts", bufs=1) as consts,
            tc.tile_pool(name="xtp", bufs=4) as xtp,
            tc.tile_pool(name="ttp", bufs=2) as ttp,
            tc.tile_pool(name="ebp", bufs=2) as ebp,
            tc.tile_pool(name="prodp", bufs=3) as prodp,
            tc.tile_pool(name="dumpp", bufs=2) as dumpp,
            tc.tile_pool(name="pt", bufs=2, space="PSUM") as pt,
            tc.tile_pool(name="ptt", bufs=1, space="PSUM") as ptt,
            tc.tile_pool(name="psc", bufs=1, space="PSUM") as psc,
        ):
            # --- constants; issued interleaved with batch 0's xt chunks so
            # nothing delays the first stage-1 matmul ---
            w_sb = consts.tile([128, NDC * A], bf16)
            u_sb = consts.tile([128, NAC * 128], bf16)
            b_sb = consts.tile([128, NAC], f32)
            num_sb = consts.tile([128, BL * NDC], f32)

            for bi in range(BL):
                S_c = sc_list[bi]
                blocks = _blocks_of(S_c)
                NBLK = len(blocks)
                starts = [sum(blocks[:i]) for i in range(NBLK)]
                S_main = min(S_c, 1024)
                has_tail = S_main < S_c

                # batch 0: split DMAs so stage 1 starts after 1/4 of the
                # data; later batches are prefetched far ahead, so a single
                # issue (~0.65us of sequencer time each) is cheaper.
                xt_t = xtp.tile([128, NDC * S_cmax], bf16)
                if bi == 0:
                    nc.sync.dma_start(
                        out=xt_t[:, :S_c], in_=xt[bi][:, :S_c]
                    )
                    nc.sync.dma_start(out=w_sb, in_=w2[:, :])
                    for lo, hi in ((1, 2), (2, 4)):
                        nc.sync.dma_start(
                            out=xt_t[:, lo * S_c : hi * S_c],
                            in_=xt[bi][:, lo * S_c : hi * S_c],
                        )
                    nc.sync.dma_start(out=b_sb, in_=b2[:, :])
                    nc.sync.dma_start(out=u_sb, in_=u2[:, :])
                else:
                    nc.sync.dma_start(
                        out=xt_t[:, : NDC * S_c], in_=xt[bi][:, : NDC * S_c]
                    )

                # stage 1 (weight-major): z^T[a, s] = W^T @ x^T, then tanh.
                tt = ttp.tile([128, NAC * S_cmax], bf16)
                for ac in range(NAC):
                    ps_big = pt.tile([128, 1024], f32, tag="pst", name="ps_big")
                    ps_tail = (
                        ptt.tile([128, 128], f32, tag="ptt", name="ps_tail")
                        if has_tail
                        else None
                    )
                    for dc in range(NDC):
                        lo = dc * A + ac * 128
                        for blk in range(NBLK):
                            st = starts[blk]
                            out_ps = (
                                ps_big[:, st : st + blocks[blk]]
                                if st < S_main
                                else ps_tail[:, : blocks[blk]]
                            )
                            nc.tensor.matmul(
                                out=out_ps,
                                lhsT=w_sb[:, lo : lo + 128],
                                rhs=xt_t[
                                    :,
                                    dc * S_c + st : dc * S_c + st + blocks[blk],
                                ],
                                start=(dc == 0),
                                stop=(dc == NDC - 1),
                            )
                    nc.scalar.activation(
                        out=tt[:, ac * S_c : ac * S_c + S_main],
                        in_=ps_big[:, :S_main],
                        func=AF.Tanh,
                        bias=b_sb[:, ac : ac + 1],
                        scale=1.0,
                    )
                    if has_tail:
                        nc.scalar.activation(
                            out=tt[:, ac * S_c + S_main : ac * S_c + S_c],
                            in_=ps_tail[:, : S_c - S_main],
                            func=AF.Tanh,
                            bias=b_sb[:, ac : ac + 1],
                            scale=1.0,
                        )

                # stage 2: score broadcast across partitions via replicated u,
                # one merged psum tile, one Exp -> e_bcast [128, S_c]
                e_b = ebp.tile([128, S_cmax], bf16)
                sc_ps = psc.tile([128, 1152], f32, tag="psc", name="sc_ps")
                for ac in range(NAC):
                    for blk in range(NBLK):
                        st = starts[blk]
                        nc.tensor.matmul(
                            out=sc_ps[:, st : st + blocks[blk]],
                            lhsT=u_sb[:, ac * 128 : (ac + 1) * 128],
                            rhs=tt[:, ac * S_c + st : ac * S_c + st + blocks[blk]],
                            start=(ac == 0),
                            stop=(ac == NAC - 1),
                        )
                nc.scalar.activation(
                    out=e_b[:, :S_c], in_=sc_ps[:, :S_c], func=AF.Exp
                )
                nc.sync.dma_start(out=eo[bi][:, :S_c], in_=e_b[0:1, :S_c])

                # weighted sum: num[d] = sum_s xt[d, s] * e[s], one fused
                # affine_mul_reduce per d-chunk (f32 accumulate). For the
                # last batch, route two chunks through ScalarE Copy+accum —
                # emitted FIRST so the drain runs on both engines in
                # parallel.
                dcs = [2, 3, 0, 1] if bi == BL - 1 else list(range(NDC))
                for dc in dcs:
                    prod = prodp.tile([128, S_cmax], bf16, tag="prod")
                    col = bi * NDC + dc
                    if bi == BL - 1 and dc >= 2:
                        nc.vector.tensor_mul(
                            out=prod[:, :S_c],
                            in0=xt_t[:, dc * S_c : dc * S_c + S_c],
                            in1=e_b[:, :S_c],
                        )
                        dump = dumpp.tile([128, S_cmax], bf16, tag="dump")
                        nc.scalar.activation(
                            out=dump[:, :S_c],
                            in_=prod[:, :S_c],
                            func=AF.Copy,
                            accum_out=num_sb[:, col : col + 1],
                        )
                    else:
                        nc.vector.affine_mul_reduce(
                            out=prod[:, :S_c],
                            accum_out=num_sb[:, col : col + 1],
                            in0=xt_t[:, dc * S_c : dc * S_c + S_c],
                            in1=e_b[:, :S_c],
                            scale=1.0,
                            bias=0.0,
                        )

            nc.sync.dma_start(out=num[:, :], in_=num_sb)

    nc.finalize()
    return nc


def _get_nc(sc_list):
    key = tuple(sc_list)
    if key not in _cache:
        _cache[key] = _build_bass(sc_list)
    return _cache[key]


def _prepare(x, mask, W, b, u):
    bf = ml_dtypes.bfloat16
    x = np.asarray(x, dtype=np.float32)
    mask = np.asarray(mask).astype(bool)

    counts = mask.sum(axis=1)

    # sort batches by count (desc); batch perm[j*NCORES + c] -> core c, slot j.
    # Slot j then needs only the max count within its band, rounded to 64.
    perm = np.argsort(-counts, kind="stable")
    sc_list = []
    for j in range(BL):
        band = counts[perm[j * NCORES : (j + 1) * NCORES]]
        mx = int(band.max())
        sc_list.append(min(S, max(256, 64 * ((mx + 63) // 64))))
    S_cmax = max(sc_list)

    # host-side compaction into the jagged packed layout:
    # xt_h[bi_slot, p, dc*S_c[j] + s] = x[batch, s_unmasked, dc*128 + p]
    xt_h = np.zeros((B, 128, NDC * S_cmax), dtype=bf)
    maskc = np.zeros((B, S_cmax), dtype=np.float32)
    for j in range(BL):
        S_c = sc_list[j]
        for c in range(NCORES):
            bidx = int(perm[j * NCORES + c])
            idx = np.flatnonzero(mask[bidx])
            xcb = np.zeros((S_c, D), dtype=np.float32)
            xcb[: idx.size] = x[bidx, idx]
            # [S_c, D] -> [128, NDC*S_c] with layout p, dc*S_c + s
            packed = (
                xcb.T.reshape(NDC, 128, S_c).transpose(1, 0, 2).reshape(128, NDC * S_c)
            )
            xt_h[c * BL + j, :, : NDC * S_c] = packed.astype(bf)
            maskc[c * BL + j, : idx.size] = 1.0

    w2_h = np.ascontiguousarray(
        np.asarray(W, dtype=np.float32).reshape(NDC, 128, A).transpose(1, 0, 2).reshape(128, NDC * A)
    ).astype(bf)
    u_col = np.asarray(u, dtype=np.float32)[:, 0].reshape(NAC, 128).T  # [128, NAC]
    u2_h = np.ascontiguousarray(
        np.repeat(u_col[:, :, None], 128, axis=2).reshape(128, NAC * 128)
    ).astype(bf)
    b2_h = np.ascontiguousarray(
        np.asarray(b, dtype=np.float32).reshape(NAC, 128).T
    ).astype(np.float32)
    return sc_list, perm, xt_h, maskc, w2_h, u2_h, b2_h


def kernel(x, mask, W, b, u):
    global last_results
    from concourse.bass_utils import run_bass_kernel_spmd

    sc_list, perm, xt_h, maskc, w2_h, u2_h, b2_h = _prepare(x, mask, W, b, u)
    nc = _get_nc(sc_list)
    in_maps = []
    for c in range(NCORES):
        sl = slice(c * BL, (c + 1) * BL)
        in_maps.append(
            {
                "xt": xt_h[sl],
                "w2": w2_h,
                "u2": u2_h,
                "b2": b2_h,
            }
        )

    try:
        res = run_bass_kernel_spmd(nc, in_maps, core_ids=list(range(NCORES)))
    except ModuleNotFoundError:
        # BASS_TRACE requested but the axon NTFF hook module is absent;
        # rerun without tracing.
        os.environ["BASS_NEVER_TRACE"] = "1"
        res = run_bass_kernel_spmd(nc, in_maps, core_ids=list(range(NCORES)))
    last_results = res

    out = np.empty((B, D), dtype=np.float32)
    for c in range(NCORES):
        num_h = res.results[c]["num"]                    # [128, BL*NDC] f32
        e_h = res.results[c]["eo"].astype(np.float32)    # [BL, 1, S_cmax]
        num_bd = (
            num_h.reshape(128, BL, NDC).transpose(1, 2, 0).reshape(BL, D)
        )
        for j in range(BL):
            bidx = int(perm[j * NCORES + c])
            sc = sc_list[j]
            den = (e_h[j, 0, :sc] * maskc[c * BL + j, :sc]).sum() + np.float32(EPS)
            out[bidx] = num_bd[j] / den
    return out.astype(np.float32)
